# revision 8
# baseline (speedup 1.0000x reference)
"""Multi-head attention (B=4, S=2048, D=1024, H=16) on 8 Trainium2 NeuronCores.

Sharding: batch x head-group. Core c handles batch c//2 and heads
[8*(c%2), 8*(c%2)+8).  Each core computes QKV projections (Megatron
column-shard), attention for its 8 heads, and a row-sharded out-projection
partial; the host sums the two partials per batch and adds b_out.

Device layouts (per core):
  xT   [1024, 2048]  x[b].T             (K on partitions for projections)
  qT/kT [128, 2048] x4 tiles            head-pair-packed, feature rows on partitions
  v    [128, 520] x16 tiles             tokens on partitions; head h's 65 cols are
                                        [vals(64) | 1] so the AV matmul emits the
                                        softmax denominator row for free
  logits are computed transposed (t on partitions) so softmax's matmuls need no
  transposes; the ones-column of v makes the AV matmul also emit the softmax
  denominator row for free.  exp runs on ACT with the 1/sqrt(64) scale folded in.
  All matmuls run as float32r (tf32-like precision, 1 cycle/row).
"""
import sys

sys.path.insert(0, "/opt/trn_rl_repo")

import numpy as np

import concourse.bass as bass
import concourse.mybir as mybir
import concourse.tile as tile
from concourse.bass_utils import run_bass_kernel_spmd

F32 = mybir.dt.float32
F32R = mybir.dt.float32r
EXP = mybir.ActivationFunctionType.Exp

DIM = 1024
S = 2048
H_PER_CORE = 8
NK = DIM // 128  # 8 k-chunks
NTB = S // 512  # 4 token blocks
NST = S // 128  # 16 s-tiles / t-chunks


def split_excess_waits(nc, maxw=1):
    """walrus (CoreV3) encodes at most one sync-wait per instruction; move
    extras onto fresh same-engine NoOps placed immediately before."""
    nid = [10 ** 6]
    for f in nc.m.functions:
        for b in f.blocks:
            il = b.instructions
            out = []
            for inst in il:
                si = inst.sync_info
                if si is not None and si.on_wait and len(si.on_wait) > maxw:
                    waits = list(si.on_wait)
                    extra, keep = waits[:-maxw], waits[-maxw:]
                    for w in extra:
                        nid[0] += 1
                        nop = mybir.InstNoOp(
                            name=f"I-waitsplit-{nid[0]}", ins=[], outs=[]
                        )
                        nop.engine = inst.engine
                        nop.sync_info = mybir.SyncInfo(on_wait=[w], on_update=[])
                        out.append(nop)
                    si.on_wait = keep
                    inst.sync_info = si
                out.append(inst)
            il[:] = out


def _pview(t, offset_elems, dims):
    """AP into tile t at free-dim element offset with explicit [stride, count]
    free dims (partition dim taken from the tile)."""
    return bass.AP(
        tensor=t.tensor,
        offset=t.offset + offset_elems,
        ap=[list(t.ap[0])] + [list(d) for d in dims],
    )


def build_attention_nc():
    nc = bass.Bass()
    xT = nc.declare_dram_parameter("xT", [DIM, S], F32R, isOutput=False)
    wq = nc.declare_dram_parameter("wq", [DIM, 512], F32R, isOutput=False)
    wk = nc.declare_dram_parameter("wk", [DIM, 512], F32R, isOutput=False)
    wv = nc.declare_dram_parameter("wv", [DIM, 520], F32R, isOutput=False)
    wo = nc.declare_dram_parameter("wo", [512, DIM], F32R, isOutput=False)
    bq = nc.declare_dram_parameter("bq", [4, 128], F32, isOutput=False)
    bk = nc.declare_dram_parameter("bk", [4, 128], F32, isOutput=False)
    bv = nc.declare_dram_parameter("bv", [520], F32, isOutput=False)
    ident = nc.declare_dram_parameter("ident", [64, 128], F32R, isOutput=False)
    out = nc.declare_dram_parameter("out", [S, DIM], F32, isOutput=True)

    with tile.TileContext(nc) as tc:
        import contextlib

        with contextlib.ExitStack() as root:
            persist = root.enter_context(tc.tile_pool(name="persist", bufs=1))
            qT = [persist.tile([128, S], F32R, tag=f"qt{m}", name=f"qt{m}") for m in range(4)]
            kT = [persist.tile([128, S], F32R, tag=f"kt{m}", name=f"kt{m}") for m in range(4)]
            vt = [persist.tile([128, 520], F32R, tag=f"v{i}", name=f"v{i}") for i in range(NST)]

            # ---------------- Phase A: QKV projections ----------------
            with contextlib.ExitStack() as pha:
                pa = pha.enter_context(tc.tile_pool(name="phA", bufs=1))
                pax = pha.enter_context(tc.tile_pool(name="phAx", bufs=16))
                psA = pha.enter_context(
                    tc.tile_pool(name="psA", bufs=4, space="PSUM")
                )
                psV = pha.enter_context(
                    tc.tile_pool(name="psV", bufs=2, space="PSUM")
                )

                wq_t = [pa.tile([128, 512], F32R, tag=f"wq{k}", name=f"wq{k}") for k in range(NK)]
                wk_t = [pa.tile([128, 512], F32R, tag=f"wk{k}", name=f"wk{k}") for k in range(NK)]
                wv_t = [pa.tile([128, 520], F32R, tag=f"wv{k}", name=f"wv{k}") for k in range(NK)]
                for k in range(NK):
                    nc.sync.dma_start(out=wq_t[k], in_=wq[128 * k:128 * k + 128, :])
                    nc.sync.dma_start(out=wk_t[k], in_=wk[128 * k:128 * k + 128, :])
                    nc.sync.dma_start(out=wv_t[k], in_=wv[128 * k:128 * k + 128, :])
                bq_t = [pa.tile([128, 1], F32, tag=f"bq{m}", name=f"bq{m}") for m in range(4)]
                bk_t = [pa.tile([128, 1], F32, tag=f"bk{m}", name=f"bk{m}") for m in range(4)]
                for m in range(4):
                    nc.sync.dma_start(
                        out=bq_t[m],
                        in_=bq[m, :].rearrange("(p one) -> p one", one=1),
                    )
                    nc.sync.dma_start(
                        out=bk_t[m],
                        in_=bk[m, :].rearrange("(p one) -> p one", one=1),
                    )
                bvb = pa.tile([128, 520], F32, tag="bvb")
                bv_ap = bv[:]
                nc.sync.dma_start(
                    out=bvb,
                    in_=bass.AP(tensor=bv_ap.tensor, offset=bv_ap.offset,
                                ap=[[0, 128], [1, 520]]),
                )

                for tb in range(NTB):
                    c0 = 512 * tb
                    xt = [pax.tile([128, 512], F32R, tag="xt", name="xt") for _ in range(NK)]
                    for k in range(NK):
                        nc.sync.dma_start(
                            out=xt[k], in_=xT[128 * k:128 * k + 128, c0:c0 + 512]
                        )
                    for m in range(4):
                        pq = psA.tile([128, 512], F32, tag="qkproj")
                        for k in range(NK):
                            nc.tensor.matmul(
                                pq, wq_t[k][:, 128 * m:128 * m + 128], xt[k],
                                start=(k == 0), stop=(k == NK - 1),
                            )
                        nc.vector.tensor_scalar_add(
                            qT[m][:, c0:c0 + 512], pq, bq_t[m][:, 0:1]
                        )
                        pk = psA.tile([128, 512], F32, tag="qkproj")
                        for k in range(NK):
                            nc.tensor.matmul(
                                pk, wk_t[k][:, 128 * m:128 * m + 128], xt[k],
                                start=(k == 0), stop=(k == NK - 1),
                            )
                        nc.vector.tensor_scalar_add(
                            kT[m][:, c0:c0 + 512], pk, bk_t[m][:, 0:1]
                        )
                    for tt in range(4):
                        vi = 4 * tb + tt
                        pv = psV.tile([128, 520], F32, tag="vproj")
                        for k in range(NK):
                            xs = xt[k][:, 128 * tt:128 * tt + 128]
                            nc.tensor.matmul(
                                pv[:, 0:512], xs, wv_t[k][:, 0:512],
                                start=(k == 0), stop=(k == NK - 1),
                            )
                            nc.tensor.matmul(
                                pv[:, 512:520], xs, wv_t[k][:, 512:520],
                                start=(k == 0), stop=(k == NK - 1),
                            )
                        # head h's vals at cols 65h..65h+64; ones col at 65h+64
                        # (wv zero col + bv 1.0 there)
                        nc.vector.tensor_add(vt[vi], pv, bvb)

            # ---------------- Phase B: attention ----------------
            with contextlib.ExitStack() as phb:
                pb = phb.enter_context(tc.tile_pool(name="phB", bufs=1))
                ppt = phb.enter_context(tc.tile_pool(name="phBpt", bufs=5))
                psmall = phb.enter_context(tc.tile_pool(name="phBs", bufs=3))
                pdram = phb.enter_context(
                    tc.tile_pool(name="phBd", bufs=3, space="DRAM")
                )
                attn_psum = phb.enter_context(contextlib.ExitStack())
                psLT = attn_psum.enter_context(
                    tc.tile_pool(name="psLT", bufs=2, space="PSUM")
                )
                psAV = attn_psum.enter_context(
                    tc.tile_pool(name="psAV", bufs=2, space="PSUM")
                )
                valsT = [pb.tile([128, S], F32R, tag=f"vals{m}", name=f"vals{m}") for m in range(4)]
                wo_t = [pb.tile([128, DIM], F32R, tag=f"wo{k}", name=f"wo{k}") for k in range(4)]
                for k in range(4):
                    nc.sync.dma_start(out=wo_t[k], in_=wo[128 * k:128 * k + 128, :])
                id_t = pb.tile([64, 128], F32R, tag="ident")
                nc.sync.dma_start(out=id_t, in_=ident[:, :])

                for h in range(H_PER_CORE):
                    p, odd = h // 2, h % 2
                    ro = 64 * odd
                    qs = qT[p][ro:ro + 64, :]
                    ks = kT[p][ro:ro + 64, :]
                    vcol = 65 * h
                    for sb in range(2):
                        s0 = 1024 * sb
                        av = psAV.tile([128, 1024], F32, tag="av")
                        avr = av[0:65, :]
                        for tck in range(NST):
                            t0 = 128 * tck
                            lt = psLT.tile([128, 1024], F32, tag="lt")
                            for half in range(2):
                                nc.tensor.matmul(
                                    lt[:, 512 * half:512 * half + 512],
                                    ks[:, t0:t0 + 128],
                                    qs[:, s0 + 512 * half:s0 + 512 * half + 512],
                                    start=True, stop=True,
                                )
                            pt = ppt.tile([128, 1024], F32R, tag="pt")
                            nc.scalar.activation(pt, lt, EXP, scale=0.125)
                            for half in range(2):
                                nc.tensor.matmul(
                                    avr[:, 512 * half:512 * half + 512],
                                    vt[tck][:, vcol:vcol + 65],
                                    pt[:, 512 * half:512 * half + 512],
                                    start=(tck == 0), stop=(tck == NST - 1),
                                )
                        drow = psmall.tile([1, 1024], F32, tag="drow")
                        nc.vector.tensor_copy(drow, av[64:65, :])
                        dscr = pdram.tile([1, 1024], F32, tag="dscr")
                        nc.sync.dma_start(out=dscr, in_=drow)
                        rec = psmall.tile([64, 1024], F32, tag="rec")
                        nc.sync.dma_start(
                            out=rec,
                            in_=bass.AP(tensor=dscr.tensor, offset=dscr.offset,
                                        ap=[[0, 64]] + [list(d) for d in dscr.ap[1:]]),
                        )
                        nc.vector.reciprocal(rec, rec)
                        if odd == 0:
                            nc.vector.tensor_mul(
                                valsT[p][0:64, s0:s0 + 1024], av[0:64, :], rec
                            )
                        else:
                            # normalize into a transient, then shift to
                            # partitions 64..127 through the PE (identity matmul)
                            tmp = psmall.tile([64, 1024], F32R, tag="oddtmp")
                            nc.vector.tensor_mul(tmp, av[0:64, :], rec)
                            pk = psAV.tile([128, 1024], F32, tag="av")
                            for half in range(2):
                                # id_t = [0 | I64]: rows 0-63 of out get zeros,
                                # rows 64-127 get tmp -- dst stays base-0
                                nc.tensor.matmul(
                                    pk[:, 512 * half:512 * half + 512],
                                    id_t,
                                    tmp[:, 512 * half:512 * half + 512],
                                    start=True, stop=True,
                                )
                            nc.vector.tensor_copy(
                                valsT[p][64:128, s0:s0 + 1024], pk[64:128, :]
                            )

                attn_psum.close()

                # ---------------- Phase C: out projection ----------------
                with contextlib.ExitStack() as phc:
                    psO = phc.enter_context(
                        tc.tile_pool(name="psO", bufs=4, space="PSUM")
                    )
                    pob = phc.enter_context(tc.tile_pool(name="phC", bufs=4))
                    for st in range(NST):
                        r0 = 128 * st
                        for nh in range(2):
                            n0 = 512 * nh
                            po = psO.tile([128, 512], F32, tag="o")
                            for kc in range(4):
                                nc.tensor.matmul(
                                    po,
                                    valsT[kc][:, r0:r0 + 128],
                                    wo_t[kc][:, n0:n0 + 512],
                                    start=(kc == 0), stop=(kc == 3),
                                )
                            ob = pob.tile([128, 512], F32, tag="ob")
                            nc.vector.tensor_copy(ob, po)
                            nc.sync.dma_start(
                                out=out[r0:r0 + 128, n0:n0 + 512], in_=ob
                            )

    split_excess_waits(nc)
    return nc


_NC_CACHE = None


def _get_nc():
    global _NC_CACHE
    if _NC_CACHE is None:
        _NC_CACHE = build_attention_nc()
    return _NC_CACHE


def make_in_maps(x, W_qkv, b_qkv, W_out):
    H, HD = 16, 64
    in_maps = []
    group_cache = {}
    for c in range(8):
        b, g = c // 2, c % 2
        if g not in group_cache:
            heads = range(8 * g, 8 * g + 8)
            qcols = np.concatenate([np.arange(192 * h, 192 * h + 64) for h in heads])
            kcols = qcols + 64
            vcols = qcols + 128
            wq = np.ascontiguousarray(W_qkv[:, qcols])
            wk = np.ascontiguousarray(W_qkv[:, kcols])
            wv_cols = np.ascontiguousarray(W_qkv[:, vcols])  # [1024, 512]
            wv = np.zeros((1024, 520), dtype=np.float32)
            for h in range(8):
                wv[:, 65 * h:65 * h + 64] = wv_cols[:, 64 * h:64 * h + 64]
            bqg = np.ascontiguousarray(b_qkv[qcols]).reshape(4, 128)
            bkg = np.ascontiguousarray(b_qkv[kcols]).reshape(4, 128)
            bvg_flat = b_qkv[vcols]  # [512] head-major
            bvg = np.zeros(520, dtype=np.float32)
            for h in range(8):
                bvg[65 * h:65 * h + 64] = bvg_flat[64 * h:64 * h + 64]
                bvg[65 * h + 64] = 1.0
            wog = np.ascontiguousarray(W_out[512 * g:512 * g + 512, :])
            group_cache[g] = (wq, wk, wv, bqg, bkg, bvg, wog)
        wq, wk, wv, bqg, bkg, bvg, wog = group_cache[g]
        in_maps.append({
            "xT": np.ascontiguousarray(x[b].T),
            "wq": wq, "wk": wk, "wv": wv,
            "bq": bqg, "bk": bkg, "bv": bvg,
            "wo": wog,
            "ident": np.concatenate([np.zeros((64, 64), np.float32),
                                     np.eye(64, dtype=np.float32)], axis=1),
        })
    return in_maps


def kernel(x, W_qkv, b_qkv, W_out, b_out):
    nc = _get_nc()
    in_maps = make_in_maps(x, W_qkv, b_qkv, W_out)
    res = run_bass_kernel_spmd(nc, in_maps, list(range(8)))
    B = x.shape[0]
    y = np.empty((B, S, DIM), dtype=np.float32)
    for b in range(B):
        y[b] = res.results[2 * b]["out"] + res.results[2 * b + 1]["out"] + b_out
    return y


# revision 11
# speedup vs baseline: 158.6505x; 158.6505x over previous
"""Multi-head attention (B=4, S=2048, D=1024, H=16) on 8 Trainium2 NeuronCores.

Sharding: batch x head-group. Core c handles batch c//2 and heads
[8*(c%2), 8*(c%2)+8).  Each core computes QKV projections (Megatron
column-shard), attention for its 8 heads, and a row-sharded out-projection
partial; the host sums the two partials per batch and adds b_out.

Device layouts (per core):
  xT   [1024, 2048]  x[b].T             (K on partitions for projections)
  qT/kT [128, 2048] x4 tiles            head-pair-packed, feature rows on partitions
  v    [128, 520] x16 tiles             tokens on partitions; head h's 65 cols are
                                        [vals(64) | 1] so the AV matmul emits the
                                        softmax denominator row for free
  logits are computed transposed (t on partitions) so softmax's matmuls need no
  transposes; the ones-column of v makes the AV matmul also emit the softmax
  denominator row for free.  exp runs on ACT with the 1/sqrt(64) scale folded in.
  All matmul operands are fp16 (fp32 PSUM accumulation) -- full PE rate,
  half the SBUF and half the host->device transfer of fp32.
"""
import sys

sys.path.insert(0, "/opt/trn_rl_repo")

import numpy as np

import concourse.bass as bass
import concourse.mybir as mybir
import concourse.tile as tile
from concourse.bass_utils import run_bass_kernel_spmd

F32 = mybir.dt.float32
F32R = mybir.dt.float32r
F16 = mybir.dt.float16
EXP = mybir.ActivationFunctionType.Exp

DIM = 1024
S = 2048
H_PER_CORE = 8
NK = DIM // 128  # 8 k-chunks
NTB = S // 512  # 4 token blocks
NST = S // 128  # 16 s-tiles / t-chunks


def split_excess_waits(nc, maxw=1):
    """walrus (CoreV3) encodes at most one sync-wait per instruction; move
    extras onto fresh same-engine NoOps placed immediately before."""
    nid = [10 ** 6]
    for f in nc.m.functions:
        for b in f.blocks:
            il = b.instructions
            out = []
            for inst in il:
                si = inst.sync_info
                if si is not None and si.on_wait and len(si.on_wait) > maxw:
                    waits = list(si.on_wait)
                    extra, keep = waits[:-maxw], waits[-maxw:]
                    for w in extra:
                        nid[0] += 1
                        nop = mybir.InstNoOp(
                            name=f"I-waitsplit-{nid[0]}", ins=[], outs=[]
                        )
                        nop.engine = inst.engine
                        nop.sync_info = mybir.SyncInfo(on_wait=[w], on_update=[])
                        out.append(nop)
                    si.on_wait = keep
                    inst.sync_info = si
                out.append(inst)
            il[:] = out


def _pview(t, offset_elems, dims):
    """AP into tile t at free-dim element offset with explicit [stride, count]
    free dims (partition dim taken from the tile)."""
    return bass.AP(
        tensor=t.tensor,
        offset=t.offset + offset_elems,
        ap=[list(t.ap[0])] + [list(d) for d in dims],
    )


def build_attention_nc():
    nc = bass.Bass()
    xT = nc.declare_dram_parameter("xT", [DIM, S], F16, isOutput=False)
    wq = nc.declare_dram_parameter("wq", [DIM, 512], F16, isOutput=False)
    wk = nc.declare_dram_parameter("wk", [DIM, 512], F16, isOutput=False)
    wv = nc.declare_dram_parameter("wv", [DIM, 520], F16, isOutput=False)
    wo = nc.declare_dram_parameter("wo", [512, DIM], F16, isOutput=False)
    bq = nc.declare_dram_parameter("bq", [4, 128], F32, isOutput=False)
    bk = nc.declare_dram_parameter("bk", [4, 128], F32, isOutput=False)
    bv = nc.declare_dram_parameter("bv", [520], F32, isOutput=False)
    ident = nc.declare_dram_parameter("ident", [64, 128], F16, isOutput=False)
    out = nc.declare_dram_parameter("out", [S, DIM], F32, isOutput=True)

    with tile.TileContext(nc) as tc:
        import contextlib

        with contextlib.ExitStack() as root:
            persist = root.enter_context(tc.tile_pool(name="persist", bufs=1))
            qT = [persist.tile([128, S], F16, tag=f"qt{m}", name=f"qt{m}") for m in range(4)]
            kT = [persist.tile([128, S], F16, tag=f"kt{m}", name=f"kt{m}") for m in range(4)]
            vt = [persist.tile([128, 520], F16, tag=f"v{i}", name=f"v{i}") for i in range(NST)]

            # ---------------- Phase A: QKV projections ----------------
            with contextlib.ExitStack() as pha:
                pa = pha.enter_context(tc.tile_pool(name="phA", bufs=1))
                pax = pha.enter_context(tc.tile_pool(name="phAx", bufs=16))
                psA = pha.enter_context(
                    tc.tile_pool(name="psA", bufs=4, space="PSUM")
                )
                psV = pha.enter_context(
                    tc.tile_pool(name="psV", bufs=2, space="PSUM")
                )

                wq_t = [pa.tile([128, 512], F16, tag=f"wq{k}", name=f"wq{k}") for k in range(NK)]
                wk_t = [pa.tile([128, 512], F16, tag=f"wk{k}", name=f"wk{k}") for k in range(NK)]
                wv_t = [pa.tile([128, 520], F16, tag=f"wv{k}", name=f"wv{k}") for k in range(NK)]
                for k in range(NK):
                    nc.sync.dma_start(out=wq_t[k], in_=wq[128 * k:128 * k + 128, :])
                    nc.sync.dma_start(out=wk_t[k], in_=wk[128 * k:128 * k + 128, :])
                    nc.sync.dma_start(out=wv_t[k], in_=wv[128 * k:128 * k + 128, :])
                bq_t = [pa.tile([128, 1], F32, tag=f"bq{m}", name=f"bq{m}") for m in range(4)]
                bk_t = [pa.tile([128, 1], F32, tag=f"bk{m}", name=f"bk{m}") for m in range(4)]
                for m in range(4):
                    nc.sync.dma_start(
                        out=bq_t[m],
                        in_=bq[m, :].rearrange("(p one) -> p one", one=1),
                    )
                    nc.sync.dma_start(
                        out=bk_t[m],
                        in_=bk[m, :].rearrange("(p one) -> p one", one=1),
                    )
                bvb = pa.tile([128, 520], F32, tag="bvb")
                bv_ap = bv[:]
                nc.sync.dma_start(
                    out=bvb,
                    in_=bass.AP(tensor=bv_ap.tensor, offset=bv_ap.offset,
                                ap=[[0, 128], [1, 520]]),
                )

                for tb in range(NTB):
                    c0 = 512 * tb
                    xt = [pax.tile([128, 512], F16, tag="xt", name="xt") for _ in range(NK)]
                    for k in range(NK):
                        nc.sync.dma_start(
                            out=xt[k], in_=xT[128 * k:128 * k + 128, c0:c0 + 512]
                        )
                    for m in range(4):
                        pq = psA.tile([128, 512], F32, tag="qkproj")
                        for k in range(NK):
                            nc.tensor.matmul(
                                pq, wq_t[k][:, 128 * m:128 * m + 128], xt[k],
                                start=(k == 0), stop=(k == NK - 1),
                            )
                        nc.vector.tensor_scalar_add(
                            qT[m][:, c0:c0 + 512], pq, bq_t[m][:, 0:1]
                        )
                        pk = psA.tile([128, 512], F32, tag="qkproj")
                        for k in range(NK):
                            nc.tensor.matmul(
                                pk, wk_t[k][:, 128 * m:128 * m + 128], xt[k],
                                start=(k == 0), stop=(k == NK - 1),
                            )
                        nc.vector.tensor_scalar_add(
                            kT[m][:, c0:c0 + 512], pk, bk_t[m][:, 0:1]
                        )
                    for tt in range(4):
                        vi = 4 * tb + tt
                        pv = psV.tile([128, 520], F32, tag="vproj")
                        for k in range(NK):
                            xs = xt[k][:, 128 * tt:128 * tt + 128]
                            nc.tensor.matmul(
                                pv[:, 0:512], xs, wv_t[k][:, 0:512],
                                start=(k == 0), stop=(k == NK - 1),
                            )
                            nc.tensor.matmul(
                                pv[:, 512:520], xs, wv_t[k][:, 512:520],
                                start=(k == 0), stop=(k == NK - 1),
                            )
                        # head h's vals at cols 65h..65h+64; ones col at 65h+64
                        # (wv zero col + bv 1.0 there)
                        nc.vector.tensor_add(vt[vi], pv, bvb)

            # ---------------- Phase B: attention ----------------
            with contextlib.ExitStack() as phb:
                pb = phb.enter_context(tc.tile_pool(name="phB", bufs=1))
                ppt = phb.enter_context(tc.tile_pool(name="phBpt", bufs=5))
                psmall = phb.enter_context(tc.tile_pool(name="phBs", bufs=3))
                pdram = phb.enter_context(
                    tc.tile_pool(name="phBd", bufs=3, space="DRAM")
                )
                attn_psum = phb.enter_context(contextlib.ExitStack())
                psLT = attn_psum.enter_context(
                    tc.tile_pool(name="psLT", bufs=2, space="PSUM")
                )
                psAV = attn_psum.enter_context(
                    tc.tile_pool(name="psAV", bufs=2, space="PSUM")
                )
                valsT = [pb.tile([128, S], F16, tag=f"vals{m}", name=f"vals{m}") for m in range(4)]
                wo_t = [pb.tile([128, DIM], F16, tag=f"wo{k}", name=f"wo{k}") for k in range(4)]
                for k in range(4):
                    nc.sync.dma_start(out=wo_t[k], in_=wo[128 * k:128 * k + 128, :])
                id_t = pb.tile([64, 128], F16, tag="ident")
                nc.sync.dma_start(out=id_t, in_=ident[:, :])

                for h in range(H_PER_CORE):
                    p, odd = h // 2, h % 2
                    ro = 64 * odd
                    qs = qT[p][ro:ro + 64, :]
                    ks = kT[p][ro:ro + 64, :]
                    vcol = 65 * h
                    for sb in range(2):
                        s0 = 1024 * sb
                        av = psAV.tile([128, 1024], F32, tag="av")
                        avr = av[0:65, :]
                        for tck in range(NST):
                            t0 = 128 * tck
                            lt = psLT.tile([128, 1024], F32, tag="lt")
                            for half in range(2):
                                nc.tensor.matmul(
                                    lt[:, 512 * half:512 * half + 512],
                                    ks[:, t0:t0 + 128],
                                    qs[:, s0 + 512 * half:s0 + 512 * half + 512],
                                    start=True, stop=True,
                                )
                            pt = ppt.tile([128, 1024], F16, tag="pt")
                            nc.scalar.activation(pt, lt, EXP, scale=0.125)
                            for half in range(2):
                                nc.tensor.matmul(
                                    avr[:, 512 * half:512 * half + 512],
                                    vt[tck][:, vcol:vcol + 65],
                                    pt[:, 512 * half:512 * half + 512],
                                    start=(tck == 0), stop=(tck == NST - 1),
                                )
                        drow = psmall.tile([1, 1024], F32, tag="drow")
                        nc.vector.tensor_copy(drow, av[64:65, :])
                        dscr = pdram.tile([1, 1024], F32, tag="dscr")
                        nc.sync.dma_start(out=dscr, in_=drow)
                        rec = psmall.tile([64, 1024], F32, tag="rec")
                        nc.sync.dma_start(
                            out=rec,
                            in_=bass.AP(tensor=dscr.tensor, offset=dscr.offset,
                                        ap=[[0, 64]] + [list(d) for d in dscr.ap[1:]]),
                        )
                        nc.vector.reciprocal(rec, rec)
                        if odd == 0:
                            nc.vector.tensor_mul(
                                valsT[p][0:64, s0:s0 + 1024], av[0:64, :], rec
                            )
                        else:
                            # normalize into a transient, then shift to
                            # partitions 64..127 through the PE (identity matmul)
                            tmp = psmall.tile([64, 1024], F16, tag="oddtmp")
                            nc.vector.tensor_mul(tmp, av[0:64, :], rec)
                            pk = psAV.tile([128, 1024], F32, tag="av")
                            for half in range(2):
                                # id_t = [0 | I64]: rows 0-63 of out get zeros,
                                # rows 64-127 get tmp -- dst stays base-0
                                nc.tensor.matmul(
                                    pk[:, 512 * half:512 * half + 512],
                                    id_t,
                                    tmp[:, 512 * half:512 * half + 512],
                                    start=True, stop=True,
                                )
                            nc.vector.tensor_copy(
                                valsT[p][64:128, s0:s0 + 1024], pk[64:128, :]
                            )

                attn_psum.close()

                # ---------------- Phase C: out projection ----------------
                with contextlib.ExitStack() as phc:
                    psO = phc.enter_context(
                        tc.tile_pool(name="psO", bufs=4, space="PSUM")
                    )
                    pob = phc.enter_context(tc.tile_pool(name="phC", bufs=4))
                    for st in range(NST):
                        r0 = 128 * st
                        for nh in range(2):
                            n0 = 512 * nh
                            po = psO.tile([128, 512], F32, tag="o")
                            for kc in range(4):
                                nc.tensor.matmul(
                                    po,
                                    valsT[kc][:, r0:r0 + 128],
                                    wo_t[kc][:, n0:n0 + 512],
                                    start=(kc == 0), stop=(kc == 3),
                                )
                            ob = pob.tile([128, 512], F32, tag="ob")
                            nc.vector.tensor_copy(ob, po)
                            nc.sync.dma_start(
                                out=out[r0:r0 + 128, n0:n0 + 512], in_=ob
                            )

    split_excess_waits(nc)
    return nc


_NC_CACHE = None


def _get_nc():
    global _NC_CACHE
    if _NC_CACHE is None:
        _NC_CACHE = build_attention_nc()
    return _NC_CACHE


def make_group_inputs(W_qkv, b_qkv, W_out, g):
    """Weight shards for head-group g (heads 8g..8g+8)."""
    heads = range(8 * g, 8 * g + 8)
    qcols = np.concatenate([np.arange(192 * h, 192 * h + 64) for h in heads])
    kcols = qcols + 64
    vcols = qcols + 128
    wq = np.ascontiguousarray(W_qkv[:, qcols]).astype(np.float16)
    wk = np.ascontiguousarray(W_qkv[:, kcols]).astype(np.float16)
    wv_cols = W_qkv[:, vcols]  # [1024, 512]
    wv = np.zeros((1024, 520), dtype=np.float16)
    bvg_flat = b_qkv[vcols]
    bvg = np.zeros(520, dtype=np.float32)
    for h in range(8):
        wv[:, 65 * h:65 * h + 64] = wv_cols[:, 64 * h:64 * h + 64]
        bvg[65 * h:65 * h + 64] = bvg_flat[64 * h:64 * h + 64]
        bvg[65 * h + 64] = 1.0
    bqg = np.ascontiguousarray(b_qkv[qcols]).reshape(4, 128)
    bkg = np.ascontiguousarray(b_qkv[kcols]).reshape(4, 128)
    wog = np.ascontiguousarray(W_out[512 * g:512 * g + 512, :]).astype(np.float16)
    ident = np.concatenate(
        [np.zeros((64, 64), np.float16), np.eye(64, dtype=np.float16)], axis=1
    )
    return {"wq": wq, "wk": wk, "wv": wv, "bq": bqg, "bk": bkg, "bv": bvg,
            "wo": wog, "ident": ident}


class _Runner:
    """Caches the jitted SPMD executable and device-resident output buffers.

    Mesh is (pair=4, half=2): device (b, g) = core 2b+g runs batch b with
    head-group g.  xT ships per-batch (replicated over `half`), weights ship
    per-group (replicated over `pair`) -- each unique byte crosses the wire
    once per replica instead of once per core pair.
    """

    def __init__(self):
        import jax
        import jax.core
        from jax.sharding import Mesh, PartitionSpec, NamedSharding
        from jax.experimental.shard_map import shard_map
        from concourse import bass2jax

        self.jax = jax
        nc = _get_nc()
        self.nc = nc
        bass2jax.install_neuronx_cc_hook()
        part = nc.partition_id_tensor.name if nc.partition_id_tensor else None
        in_names, out_names, out_avals, zero_outs = [], [], [], []
        for alloc in nc.m.functions[0].allocations:
            if not isinstance(alloc, mybir.MemoryLocationSet):
                continue
            name = alloc.memorylocations[0].name
            if alloc.kind == "ExternalInput":
                if name != part:
                    in_names.append(name)
            elif alloc.kind == "ExternalOutput":
                np_dt = mybir.dt.np(alloc.dtype)
                out_names.append(name)
                out_avals.append(jax.core.ShapedArray(tuple(alloc.tensor_shape), np_dt))
                zero_outs.append(np.zeros(tuple(alloc.tensor_shape), np_dt))
        self.in_names = in_names
        n_params, n_outs = len(in_names), len(out_names)
        all_names = list(in_names) + list(out_names)
        if part is not None:
            all_names.append(part)

        def _body(*args):
            operands = list(args)
            if part is not None:
                operands.append(bass2jax.partition_id_tensor())
            outs = bass2jax._bass_exec_p.bind(
                *operands,
                out_avals=tuple(out_avals),
                in_names=tuple(all_names),
                out_names=tuple(out_names),
                lowering_input_output_aliases=(),
                sim_require_finite=True,
                sim_require_nnan=True,
                nc=nc,
            )
            return tuple(outs)

        devices = jax.devices()[:8]
        mesh = Mesh(np.asarray(devices).reshape(4, 2), ("pair", "half"))
        by_pair = {"xT"}
        in_specs = tuple(
            [PartitionSpec("pair") if nm in by_pair else PartitionSpec("half")
             for nm in in_names]
            + [PartitionSpec(("pair", "half"))] * n_outs
        )
        out_specs = (PartitionSpec(("pair", "half")),) * n_outs
        self.sharded = jax.jit(
            shard_map(_body, mesh=mesh, in_specs=in_specs,
                      out_specs=out_specs, check_rep=False),
            keep_unused=True,
        )
        self.in_shardings = [
            NamedSharding(mesh, s) for s in in_specs[:n_params]
        ]
        zsh = NamedSharding(mesh, PartitionSpec(("pair", "half")))
        self.dev_zeros = [
            jax.device_put(np.zeros((8 * z.shape[0], *z.shape[1:]), z.dtype), zsh)
            for z in zero_outs
        ]
        jax.block_until_ready(self.dev_zeros)

    def global_inputs(self, x, W_qkv, b_qkv, W_out):
        g0 = make_group_inputs(W_qkv, b_qkv, W_out, 0)
        g1 = make_group_inputs(W_qkv, b_qkv, W_out, 1)
        glob = {"xT": np.ascontiguousarray(
            x.transpose(0, 2, 1).reshape(4 * DIM, S)).astype(np.float16)}
        for nm in self.in_names:
            if nm != "xT":
                glob[nm] = np.concatenate([g0[nm], g1[nm]], axis=0)
        return [glob[nm] for nm in self.in_names]

    def run(self, x, W_qkv, b_qkv, W_out):
        concat_in = self.global_inputs(x, W_qkv, b_qkv, W_out)
        out_arrs = self.sharded(*concat_in, *self.dev_zeros)
        return np.asarray(out_arrs[0]).reshape(8, S, DIM)


_RUNNER = None


def _get_runner():
    global _RUNNER
    if _RUNNER is None:
        _RUNNER = _Runner()
    return _RUNNER


def kernel(x, W_qkv, b_qkv, W_out, b_out):
    r = _get_runner()
    o = r.run(np.asarray(x), np.asarray(W_qkv), np.asarray(b_qkv), np.asarray(W_out))
    B = x.shape[0]
    y = np.empty((B, S, DIM), dtype=np.float32)
    for b in range(B):
        y[b] = o[2 * b] + o[2 * b + 1] + b_out
    return y
